# revision 21
# baseline (speedup 1.0000x reference)
"""Multi-head attention (B=8, N=1024, D=768, H=12) on 8 TRN2 NeuronCores.

Data-parallel: one batch element per core, no collectives. Per-core kernel,
restructured for ACT/PE co-saturation:

  Pre-loop: x -> xT (PE transpose); ALL SIX qkT head-pairs precomputed
  (frees PSUM for the steady state); scores(0)+exp(0) overlap the qkT
  build; v = x @ w_v last.
  Steady loop (p = 0..4): per key-chunk interleave of scores(p+1)
  [2 matmuls -> one [128,1024] exp spanning 2 PSUM banks] with attnv(p)
  par0; then attnv(p) par1 sweep. The exp stream on the Scalar engine
  runs near-continuously; qkT bias lives on DVE (pre-loop only), the
  softmax normalize multiply/add on GpSimd, so no engine queues behind
  another's chain.
  Normalize (pairs 0-4): ones-column sums -> DRAM-roundtrip repartition
  -> reciprocal -> broadcast (throughput path, off critical chain).
  Pair 5 (tail): PE-path normalize (transpose sums, reciprocal,
  PE-broadcast) -- no DMA hops on the closing critical path.
  Proj: two 4-token-block waves, aT5 contraction last.

All matmul operands bf16 (PSUM fp32, softmax fp32).
"""

import os

import numpy as np

import bass_rust
from bass_rust import ScopedClock

import concourse.bass as bass
import concourse.tile as tile
from concourse import mybir
from concourse.bass_utils import run_bass_kernel_spmd
from concourse.masks import make_identity

# ---------------------------------------------------------------------------
# Workarounds: this container's walrus allows only ONE sync wait per
# instruction ("Too many sync wait commands"). Split extras onto same-engine
# NoOps (engine sequencers execute in program order).
# ---------------------------------------------------------------------------
_MAX_WAITS = 1


def _patched_drain_and_barrier(self, tick_clock, wait_clock):
    nc = self.nc
    drain_inst = nc.sync.drain()
    wait_clock.add_sem_waits(
        drain_inst.ins, ScopedClock({None: tick_clock.global_clock})
    )
    waits = list(drain_inst.ins.sync_info.on_wait)
    if len(waits) > _MAX_WAITS:
        drain_inst.ins.sync_info = bass_rust.SyncInfo(on_wait=[], on_update=[])
        by_num = {h.num: h for h in self.sems.allocated().values()}
        for w in waits:
            h = by_num.get(w.id)
            if h is None:
                h = bass_rust.SemaphoreHandle(name=w.ant_name, num=w.id)
            nc.sync.wait_ge(h, w.wait_value)

    nc.all_engine_barrier()
    assert self.sems is not None
    popped = nc._tile_sem_poison_stack.pop()
    assert popped is self._sem_poison
    nc.clear_and_free_semaphores(list(self.sems.allocated().values()))
    nc.all_engine_barrier()


tile.TileContext._drain_and_barrier = _patched_drain_and_barrier


def _legalize_waits(nc):
    n_split = 0
    for fn in nc.m.functions:
        for bb in fn.blocks:
            insts = bb.instructions
            if not any(
                i.sync_info is not None and len(i.sync_info.on_wait) > _MAX_WAITS
                for i in insts
            ):
                continue
            new = []
            for inst in insts:
                si = inst.sync_info
                if si is not None and len(si.on_wait) > _MAX_WAITS:
                    waits = list(si.on_wait)
                    keep, extra = waits[:_MAX_WAITS], waits[_MAX_WAITS:]
                    for j, w in enumerate(extra):
                        nop = mybir.InstNoOp(
                            name=f"{inst.name}-ws{j}", ins=[], outs=[],
                            engine=inst.engine,
                        )
                        nop.sync_info = bass_rust.SyncInfo(on_wait=[w], on_update=[])
                        new.append(nop)
                        n_split += 1
                    inst.sync_info = bass_rust.SyncInfo(
                        on_wait=keep, on_update=list(si.on_update)
                    )
                new.append(inst)
            bb.instructions = new
    return n_split


def _dedupe_ldweights(nc):
    """Drop an InstLdweights when the PE already holds identical weights
    (same physical AP + mode). bass emits one LDW per matmul; back-to-back
    matmuls sharing a stationary operand reload it pointlessly, and the
    reload blocks the next matmul's issue. Runs post-schedule, before
    _legalize_waits (sync_info is migrated to a NoOp)."""
    n = 0
    for fn in nc.m.functions:
        for bb in fn.blocks:
            insts = bb.instructions
            last_key = None
            new = []
            changed = False
            for inst in insts:
                tn = type(inst).__name__
                if tn == "InstLdweights":
                    key = (
                        repr(inst.ins[0]),
                        getattr(inst, "perf_mode", None),
                        getattr(inst, "is_transpose", None),
                        getattr(inst, "tile_position", None),
                    )
                    if key == last_key:
                        si = inst.sync_info
                        if si is not None and (si.on_wait or si.on_update):
                            nop = mybir.InstNoOp(
                                name=inst.name + "-dw", ins=[], outs=[],
                                engine=inst.engine,
                            )
                            nop.sync_info = si
                            new.append(nop)
                        n += 1
                        changed = True
                        continue
                    last_key = key
                new.append(inst)
            if changed:
                bb.instructions = new
    return n


# ---------------------------------------------------------------------------
# Kernel builder (per-core shapes hardcoded: x [1024, 768])
# ---------------------------------------------------------------------------
N, D, H, HD = 1024, 768, 12, 64
NT = N // 128       # 8 token chunks
DC = D // 128       # 6 d chunks (also head pairs)
CT = (2 * D) // 128  # 12 qk col tiles
KC = N // 128       # 8 key chunks
NP = H // 2         # 6 head pairs
SCALE = HD ** -0.5

F32 = mybir.dt.float32
BF16 = mybir.dt.bfloat16
Exp = mybir.ActivationFunctionType.Exp
ADD = mybir.AluOpType.add
MULT = mybir.AluOpType.mult


def build(legalize=True):
    nc = bass.Bass()
    x_d = nc.declare_dram_parameter("x", [N, D], F32, isOutput=False)
    wqkv_d = nc.declare_dram_parameter("w_qkv", [D, 3 * D], F32, isOutput=False)
    bqkv_d = nc.declare_dram_parameter("b_qkv", [3 * D], F32, isOutput=False)
    wp_d = nc.declare_dram_parameter("w_proj", [D, D], F32, isOutput=False)
    bp_d = nc.declare_dram_parameter("b_proj", [D], F32, isOutput=False)
    out_d = nc.declare_dram_parameter("out", [N, D], F32, isOutput=True)

    with tile.TileContext(nc) as tc:
        with (
            tc.tile_pool(name="persist", bufs=1) as persist,
            tc.tile_pool(name="consts", bufs=1) as consts,
            tc.tile_pool(name="pstage", bufs=2) as pstage,
            tc.tile_pool(name="p2", bufs=2) as p2,
            tc.tile_pool(name="drp", bufs=2, space="DRAM") as drp,
        ):
            qkT = persist.tile([128, CT, N], BF16)         # [qk col, tok]
            vpack = persist.tile([128, KC, NP, 2, 128], BF16)
            aT = persist.tile([128, DC - 1, N], BF16)      # [d, tok] pairs 0-4
            aT5 = persist.tile([128, N], BF16)             # [d, tok] pair 5
            wp_sb = persist.tile([128, DC, D], BF16)
            xT = persist.tile([128, DC, NT, 128], BF16)    # [d, tok]
            wqk = persist.tile([128, DC, 3 * D], BF16)
            bqk_sb = consts.tile([128, CT], F32)
            bv_sb = consts.tile([128, DC], F32)
            bout_bc = consts.tile([128, D], F32)
            ident = consts.tile([128, 128], BF16)
            ident32 = consts.tile([128, 128], F32)
            onesel = consts.tile([8, N], BF16)

            make_identity(nc, ident[:])
            make_identity(nc, ident32[:])
            # one-hot block selector: onesel[k, c*128+j] = (k == c):
            # zero-fill, then DMA a ones row into each diagonal block.
            ones128 = consts.tile([1, 128], BF16)
            nc.vector.memset(ones128[:], 1.0)
            nc.gpsimd.memset(onesel[:], 0.0)
            for k in range(8):
                nc.gpsimd.dma_start(
                    onesel[k : k + 1, k * 128 : (k + 1) * 128], ones128[:]
                )
            # ones columns of vpack (even head: col 127; odd head: col 0).
            # Unused columns stay uninitialized: their PSUM output
            # partitions are never read.
            nc.gpsimd.memset(vpack[:, :, :, 0, 64:72], 0.0)
            nc.gpsimd.memset(vpack[:, :, :, 1, 0:8], 0.0)
            nc.gpsimd.memset(vpack[:, :, :, 0, 64:65], 1.0)
            nc.gpsimd.memset(vpack[:, :, :, 1, 0:1], 1.0)

            nc.scalar.dma_start(
                bqk_sb[:], bqkv_d.ap()[: 2 * D].rearrange("(o i) -> i o", i=128)
            )
            nc.scalar.dma_start(
                bv_sb[:], bqkv_d.ap()[2 * D :].rearrange("(o i) -> i o", i=128)
            )
            bp_ap = bp_d.ap()
            nc.gpsimd.dma_start(
                bout_bc[:],
                bass.AP(tensor=bp_ap.tensor, offset=bp_ap.offset,
                        ap=[[0, 128]] + bp_ap.ap),
            )

            # ---- x load + cast (DVE) ----
            xbfs = []
            for i in range(NT):
                xst = pstage.tile([128, 1024], F32, tag="wstB",
                                  name="xst")[:, :D]
                nc.sync.dma_start(xst[:], x_d.ap()[i * 128 : (i + 1) * 128, :])
                xbf = pstage.tile([128, D], BF16, tag="xbf")
                nc.vector.tensor_copy(xbf[:], xst[:])
                xbfs.append(xbf)

            # ---- w wave 1: q+k cols of pairs 0,1 (ct 0,1,6,7), Scalar copies
            WQK = ((0, 256, 0), (768, 1024, 256))
            for i in range(DC):
                wstA = pstage.tile([128, 1024], F32, tag="wstB",
                                   name="wstA")[:, :512]
                srcw = wqkv_d.ap()[i * 128 : (i + 1) * 128, :]
                nc.sync.dma_start(
                    wstA[:].rearrange("p (g c) -> p g c", c=256),
                    bass.AP(tensor=srcw.tensor, offset=srcw.offset,
                            ap=[srcw.ap[0], [768, 2], [1, 256]]),
                )
                for c0, c1, p0 in WQK:
                    nc.scalar.copy(
                        wqk[:, i, c0:c1], wstA[:, p0 : p0 + (c1 - c0)]
                    )

            # ---- remaining w DMA issues (copies emitted later) ----
            W2 = ((256, 768, 0), (1024, 1536, 512))
            wstB_tiles = []
            for kc in range(DC):
                wstB = pstage.tile([128, 1024], F32, tag="wstB", name="wstB")
                srcw = wqkv_d.ap()[kc * 128 : (kc + 1) * 128, :]
                nc.sync.dma_start(
                    wstB[:].rearrange("p (g c) -> p g c", c=512),
                    bass.AP(tensor=srcw.tensor, offset=srcw.offset + 256,
                            ap=[srcw.ap[0], [768, 2], [1, 512]]),
                )
                wstB_tiles.append(wstB)
            wstV_tiles = []
            for kc in range(DC):
                wstV = pstage.tile([128, D], F32, tag="wstV", name="wstV")
                nc.scalar.dma_start(
                    wstV[:], wqkv_d.ap()[kc * 128 : (kc + 1) * 128, 2 * D :]
                )
                wstV_tiles.append(wstV)
            wstP_tiles = []
            for kc in range(DC):
                wstP = pstage.tile([128, D], F32, tag="wstV", name="wstP")
                nc.gpsimd.dma_start(
                    wstP[:], wp_d.ap()[kc * 128 : (kc + 1) * 128, :]
                )
                wstP_tiles.append(wstP)

            # ---- transposes x -> xT ----
            with tc.tile_pool(name="ps_tp", bufs=2, space="PSUM") as ps_tp:
                for t in range(NT):
                    for dc in range(DC):
                        tp = ps_tp.tile([128, 128], BF16, tag="tp")
                        nc.tensor.transpose(
                            tp[:], xbfs[t][:, dc * 128 : (dc + 1) * 128], ident[:]
                        )
                        nc.vector.tensor_copy(xT[:, dc, t, :], tp[:])

            expT_of = {}   # (pair, par) -> tile

            with tc.tile_pool(name="ps_j", bufs=1, space="PSUM") as ps_j:
                junk_ps = ps_j.tile([128, 128], F32, tag="junk", name="junk")

                def warm():
                    nc.tensor.matmul(
                        junk_ps[:], ident[:], ident[:],
                        start=True, stop=True, skip_group_check=True,
                    )

                with tc.tile_pool(name="ring", bufs=5, space="PSUM") as ring:

                    def emit_scores_unit(p, kc, par):
                        off = 64 * par
                        eT = expT_of[(p, par)]
                        for nh in range(2):
                            s1 = ring.tile([128, 512], F32, tag="rg",
                                           name="s1")
                            nc.tensor.matmul(
                                s1[:],
                                qkT[off : off + 64, CT // 2 + p,
                                    kc * 128 : (kc + 1) * 128],
                                qkT[off : off + 64, p,
                                    nh * 512 : (nh + 1) * 512],
                                start=True,
                                stop=True,
                            )
                            nc.scalar.activation(
                                eT[:, kc, nh * 512 : (nh + 1) * 512],
                                s1[:], Exp,
                            )

                    def alloc_expT(p, par):
                        expT_of[(p, par)] = p2.tile(
                            [128, KC, N], BF16, tag="expT", bufs=3,
                            name="expT",
                        )

                    # W2 staging copies first (qkT(2+) needs all kc);
                    # wv on Scalar, wp on gpsimd (consumed later).
                    for kc in range(DC):
                        wstB = wstB_tiles[kc]
                        (c0, c1, p0), (d0, d1, q0) = W2
                        nc.vector.tensor_copy(
                            wqk[:, kc, c0:c1], wstB[:, p0 : p0 + (c1 - c0)]
                        )
                        nc.gpsimd.tensor_copy(
                            wqk[:, kc, d0:d1], wstB[:, q0 : q0 + (d1 - d0)]
                        )

                    with tc.tile_pool(name="qk2", bufs=2,
                                      space="PSUM") as qk2p:

                        def emit_qkT_chunk(ct):
                            for nh in range(2):
                                q2 = qk2p.tile([128, 512], F32, tag="qk",
                                               name="q2")
                                for kc in range(DC):
                                    nc.tensor.matmul(
                                        q2[:],
                                        wqk[:, kc, ct * 128 : (ct + 1) * 128],
                                        xT[:, kc, nh * 4 : (nh + 1) * 4, :],
                                        start=(kc == 0),
                                        stop=(kc == DC - 1),
                                    )
                                dst = qkT[:, ct, nh * 512 : (nh + 1) * 512]
                                if ct < CT // 2:
                                    nc.vector.tensor_scalar(
                                        dst, q2[:], bqk_sb[:, ct : ct + 1],
                                        SCALE, op0=ADD, op1=MULT,
                                    )
                                else:
                                    nc.vector.tensor_scalar(
                                        dst, q2[:], bqk_sb[:, ct : ct + 1],
                                        None, op0=ADD,
                                    )

                        # ---- qkT pair 0 + scores(0) ----
                        emit_qkT_chunk(0)
                        emit_qkT_chunk(6)
                        alloc_expT(0, 0)
                        alloc_expT(0, 1)
                        for kc in range(KC):
                            emit_scores_unit(0, kc, 0)
                            emit_scores_unit(0, kc, 1)
                            warm()
                        for hp in range(1, NP):
                            emit_qkT_chunk(hp)
                            emit_qkT_chunk(CT // 2 + hp)
                        for kc in range(DC):
                            nc.scalar.copy(
                                wqk[:, kc, 2 * D :], wstV_tiles[kc][:]
                            )
                            nc.gpsimd.tensor_copy(
                                wp_sb[:, kc, :], wstP_tiles[kc][:]
                            )
                    # ---- v = x @ w_v ----
                    with tc.tile_pool(name="ps_v", bufs=1, space="PSUM") as ps_v:
                        for t in range(NT):
                            pv = ps_v.tile([128, D], F32, tag="v")
                            for kc in range(DC):
                                for j0, j1 in ((0, 512), (512, D)):
                                    nc.tensor.matmul(
                                        pv[:, j0:j1],
                                        xT[:, kc, t, :],
                                        wqk[:, kc, 2 * D + j0 : 2 * D + j1],
                                        start=(kc == 0),
                                        stop=(kc == DC - 1),
                                    )
                            psv = pv.rearrange(
                                "p (hp two c) -> p hp two c", two=2, c=64
                            )
                            nc.vector.tensor_copy(
                                vpack[:, t, :, 0, 0:64], psv[:, :, 0, :]
                            )
                            # second copy on Scalar (idle here) so the
                            # bufs=1 v-psum recycles faster
                            nc.scalar.copy(
                                vpack[:, t, :, 1, 64:128], psv[:, :, 1, :]
                            )

                        # bout = b_proj + b_v^T @ w_proj, broadcast to all
                        # 128 partitions (folds the softmax +b_v into the
                        # proj bias: attn rows sum to 1).
                        bvb = p2.tile([128, DC], BF16, tag="bvb", name="bvb")
                        nc.vector.tensor_copy(bvb[:], bv_sb[:])
                        brow_ps = ps_v.tile([1, D], F32, tag="v",
                                            name="brow_ps")
                        for kc in range(DC):
                            for j0, j1 in ((0, 512), (512, D)):
                                nc.tensor.matmul(
                                    brow_ps[:, j0:j1],
                                    bvb[:, kc : kc + 1],
                                    wp_sb[:, kc, j0:j1],
                                    start=(kc == 0),
                                    stop=(kc == DC - 1),
                                )
                        brow = p2.tile([1, D], BF16, tag="brow", name="brow")
                        nc.scalar.copy(brow[:], brow_ps[:])
                        bb_ps = ps_v.tile([128, D], F32, tag="v",
                                          name="bb_ps")
                        for j0, j1 in ((0, 512), (512, D)):
                            nc.tensor.matmul(
                                bb_ps[:, j0:j1],
                                ones128[:],
                                brow[:, j0:j1],
                                start=True,
                                stop=True,
                                skip_group_check=True,
                            )
                        nc.vector.tensor_tensor(
                            bout_bc[:], bb_ps[:], bout_bc[:], ADD
                        )

                    # ---- steady loop: iters 0..4 ----
                    def emit_chain_dma(p, par, pa, bc):
                        """pa [128,N] psum -> aT[., p] via DRAM-roundtrip
                        reciprocal broadcast. Multiply/add on gpsimd."""
                        off = 64 * par
                        sumrow = 64 if par == 0 else 0
                        pa_sb = p2.tile([128, N], F32, tag="pa_sb",
                                        name="pa_sb")
                        nc.vector.tensor_copy(pa_sb[:], pa[:])
                        rdram = drp.tile([1, N], F32, tag="rd", name="rd")
                        nc.sync.dma_start(
                            rdram[:], pa_sb[sumrow : sumrow + 1, :]
                        )
                        rp = p2.tile([128, N // 128], F32, tag="rp", name="rp")
                        nc.sync.dma_start(
                            rp[:], rdram[0].rearrange("(o i) -> i o", i=128)
                        )
                        rp2 = p2.tile([128, N // 128], F32, tag="rp2",
                                      name="rp2")
                        nc.vector.reciprocal(rp2[:], rp[:])
                        rdram2 = drp.tile([1, N], F32, tag="rd2", name="rd2")
                        nc.sync.dma_start(
                            rdram2[0].rearrange("(o i) -> i o", i=128), rp2[:]
                        )
                        rd_ap = rdram2[:]
                        rec_bcast = bass.AP(
                            tensor=rd_ap.tensor, offset=rd_ap.offset,
                            ap=[[0, 64]] + rd_ap.ap[1:],
                        )
                        nc.sync.dma_start(bc[off : off + 64, :], rec_bcast)
                        dst = aT[off : off + 64, p, :]
                        nc.gpsimd.tensor_tensor(
                            dst, pa_sb[off : off + 64, :],
                            bc[off : off + 64, :], MULT,
                        )

                    with tc.tile_pool(name="ps_a", bufs=1, space="PSUM") as ps_a:
                        for p in range(NP - 1):
                            bc = p2.tile([128, N], F32, tag="bc", name="bc")
                            for par in range(2):
                                # expT ring (bufs=3): (p+1, par) reuses the
                                # slot attnv(p-1, ~) released; par1 alloc sits
                                # after attnv(p, par0), keeping 3 live max.
                                alloc_expT(p + 1, par)
                                pa = ps_a.tile([128, N], F32, tag="pa",
                                               name="pa")
                                eT = expT_of[(p, par)]
                                for kc in range(KC):
                                    emit_scores_unit(p + 1, kc, par)
                                    for nh in range(2):
                                        nc.tensor.matmul(
                                            pa[:, nh * 512 : (nh + 1) * 512],
                                            vpack[:, kc, p, par, :],
                                            eT[:, kc,
                                               nh * 512 : (nh + 1) * 512],
                                            start=(kc == 0),
                                            stop=(kc == KC - 1),
                                        )
                                    warm()
                                emit_chain_dma(p, par, pa, bc)

                # ---- tail: pair 5 attnv + PE-path normalize ----
                with tc.tile_pool(name="ps_t", bufs=1, space="PSUM") as ps_t:
                    p = NP - 1
                    for par in range(2):
                        off = 64 * par
                        sumrow = 64 if par == 0 else 0
                        pa = ps_t.tile([128, N], F32, tag="pa5", name="pa5")
                        eT = expT_of[(p, par)]
                        for kc in range(KC):
                            for nh in range(2):
                                nc.tensor.matmul(
                                    pa[:, nh * 512 : (nh + 1) * 512],
                                    vpack[:, kc, p, par, :],
                                    eT[:, kc, nh * 512 : (nh + 1) * 512],
                                    start=(kc == 0),
                                    stop=(kc == KC - 1),
                                )
                            warm()
                        pa_sb = p2.tile([128, N], F32, tag="pa_sb",
                                        name="pa_sb")
                        nc.vector.tensor_copy(pa_sb[:], pa[:])
                        # sums row [1,N] -> [128,8] via PE transposes
                        tp_ps = ps_t.tile([128, N // 128], F32, tag="tp5",
                                          name="tp5")
                        for c in range(N // 128):
                            # [1,128] -> [128,1]: lhsT.T @ [[1.0]]
                            nc.tensor.matmul(
                                tp_ps[:, c : c + 1],
                                pa_sb[sumrow : sumrow + 1,
                                      c * 128 : (c + 1) * 128],
                                ident32[sumrow : sumrow + 1,
                                        sumrow : sumrow + 1],
                                is_transpose=True,
                                skip_group_check=True,
                            )
                        rp2 = p2.tile([128, N // 128], F32, tag="rp2",
                                      name="rp2")
                        nc.vector.reciprocal(rp2[:], tp_ps[:])
                        rp2b = p2.tile([128, N // 128], BF16, tag="rp2b",
                                       name="rp2b")
                        nc.vector.tensor_copy(rp2b[:], rp2[:])
                        rpT_ps = ps_t.tile([N // 128, 128], BF16, tag="rpT",
                                           name="rpT")
                        nc.tensor.transpose(rpT_ps[:], rp2b[:], ident[:])
                        rpT = p2.tile([N // 128, 128], BF16, tag="rpTs",
                                      name="rpTs")
                        nc.vector.tensor_copy(rpT[:], rpT_ps[:])
                        bc_ps = ps_t.tile([128, N], F32, tag="bc5", name="bc5")
                        for c in range(N // 128):
                            nc.tensor.matmul(
                                bc_ps[:, c * 128 : (c + 1) * 128],
                                onesel[:, c * 128 : (c + 1) * 128],
                                rpT[:],
                                start=True,
                                stop=True,
                                skip_group_check=True,
                            )
                        dst = aT5[off : off + 64, :]
                        nc.vector.tensor_tensor(
                            dst, pa_sb[off : off + 64, :],
                            bc_ps[off : off + 64, :], MULT,
                        )

            # ---------------- proj ----------------
            with (
                tc.tile_pool(name="p3", bufs=2) as p3,
                tc.tile_pool(name="ps_p", bufs=4, space="PSUM") as ps_p,
            ):
                for wave in (range(0, 4), range(4, NT)):
                    pps = []
                    for qt in wave:
                        pp = ps_p.tile([128, D], F32, tag="pp")
                        for kc in range(DC - 1):
                            for j0, j1 in ((0, 512), (512, D)):
                                nc.tensor.matmul(
                                    pp[:, j0:j1],
                                    aT[:, kc, qt * 128 : (qt + 1) * 128],
                                    wp_sb[:, kc, j0:j1],
                                    start=(kc == 0),
                                    stop=False,
                                )
                        pps.append((qt, pp))
                    for qt, pp in pps:
                        for j0, j1 in ((0, 512), (512, D)):
                            nc.tensor.matmul(
                                pp[:, j0:j1],
                                aT5[:, qt * 128 : (qt + 1) * 128],
                                wp_sb[:, DC - 1, j0:j1],
                                start=False,
                                stop=True,
                            )
                        ob = p3.tile([128, D], F32, tag="ob")
                        nc.vector.tensor_tensor(ob[:], pp[:], bout_bc[:], ADD)
                        nc.sync.dma_start(
                            out_d.ap()[qt * 128 : (qt + 1) * 128, :], ob[:]
                        )

    if legalize:
        if bool(int(os.environ.get("KERNEL_DEDUPE_LDW", "0"))):
            _dedupe_ldweights(nc)
        _legalize_waits(nc)
    return nc


_NC_CACHE = {}
LAST_RESULT = None


def kernel(x, w_qkv, b_qkv, w_proj, b_proj):
    global LAST_RESULT
    x = np.ascontiguousarray(np.asarray(x, dtype=np.float32))
    w_qkv = np.ascontiguousarray(np.asarray(w_qkv, dtype=np.float32))
    b_qkv = np.ascontiguousarray(np.asarray(b_qkv, dtype=np.float32))
    w_proj = np.ascontiguousarray(np.asarray(w_proj, dtype=np.float32))
    b_proj = np.ascontiguousarray(np.asarray(b_proj, dtype=np.float32))
    B = x.shape[0]
    assert x.shape == (B, N, D) and B == 8

    if "nc" not in _NC_CACHE:
        _NC_CACHE["nc"] = build()
    nc = _NC_CACHE["nc"]

    in_maps = [
        {"x": x[i], "w_qkv": w_qkv, "b_qkv": b_qkv,
         "w_proj": w_proj, "b_proj": b_proj}
        for i in range(B)
    ]
    trace = bool(int(os.environ.get("KERNEL_TRACE", "0")))
    res = run_bass_kernel_spmd(
        nc, in_maps, core_ids=list(range(8)), trace=trace
    )
    LAST_RESULT = res
    return np.stack([res.results[i]["out"] for i in range(B)], axis=0)
